# revision 1
# baseline (speedup 1.0000x reference)
"""Trainium2 Bass kernel for nn_Attention3DLayer (additive attention).

Per batch b (8 batches, one per NeuronCore):
  F   = source_b @ wk + wq @ query_b.T          (S, Q)
  G   = tanh(F)
  sim = G @ kn                                  (S, T)
  P   = softmax(sim, axis=-1)
  out = P.T @ source_b                          (Q, D)

Data-parallel over batch across 8 cores; weights replicated.

Layout strategy (per core):
  phase 1 computes F.T (Q-part, S-free) so tanh output G.T is directly the
  stationary operand for phase 2; softmax is then a free-dim reduction and
  phase 3 consumes P and native source with no further transposes.
  All matmul operands are cast to bf16 (fp32 PSUM accumulation).
"""

import os
import sys
from contextlib import ExitStack

for _p in ("/root/.axon_site/_ro/trn_rl_repo", "/opt/trn_rl_repo"):
    if os.path.isdir(_p) and _p not in sys.path:
        sys.path.append(_p)

import numpy as np

import concourse.bass as bass  # noqa: E402
import concourse.mybir as mybir  # noqa: E402
import concourse.tile as tile  # noqa: E402
from concourse import bacc, bass_utils  # noqa: E402

B, S, Q, D = 8, 2048, 2048, 1024
T = Q
P = 128
NS = S // P   # 16 s-chunks
NQ = Q // P   # 16 q-chunks
NJ = D // P   # 8 d-chunks
NT = T // P   # 16 t-tiles
NB = 512      # matmul moving free dim / PSUM bank tile
N_CORES = 8

BF16 = mybir.dt.bfloat16
F32 = mybir.dt.float32
AFT = mybir.ActivationFunctionType
AX = mybir.AxisListType


def _emit(tc, nc, src_d, qry_d, kn_d, wk_d, wq_d, out_d):
    with ExitStack() as ctx:
        gt_pool = ctx.enter_context(tc.tile_pool(name="gt_pool", bufs=1))
        # G.T: [p, qb, s] = tanh(F)[s, qb*P+p].T -> (q-part, s-free)
        gt = gt_pool.tile([P, NQ, S], BF16, name="gt")

        # ---------------- phase 1: F.T = wk.T @ src.T + query @ wq.T ------
        with ExitStack() as ph1:
            res1 = ph1.enter_context(tc.tile_pool(name="res1", bufs=1))
            stage = ph1.enter_context(tc.tile_pool(name="stage", bufs=4))
            qtp = ph1.enter_context(tc.tile_pool(name="qtp", bufs=3))
            kpre = ph1.enter_context(tc.tile_pool(name="kpre", bufs=2))
            dramp = ph1.enter_context(
                tc.tile_pool(name="dramp", bufs=1, space="DRAM")
            )
            ps1 = ph1.enter_context(
                tc.tile_pool(name="ps1", bufs=8, space="PSUM")
            )

            # kn pre-cast to bf16 in DRAM (phase 2 then loads 8MiB not 16MiB)
            kn_bf = dramp.tile([S, T], BF16, name="kn_bf")

            # wk native (d-part): wkt[p, j, q] = wk[j*P+p, q]
            wkt = res1.tile([P, NJ, Q], BF16, name="wkt")
            for j in range(NJ):
                nc.gpsimd.dma_start(wkt[:, j, :], wk_d[j * P : (j + 1) * P, :])

            # srcT[p, j, s] = src[s, j*P+p];  wqT[p, j, s] = wq[s, j*P+p]
            srcT = res1.tile([P, NJ, S], BF16, name="srcT")
            wqT = res1.tile([P, NJ, S], BF16, name="wqT")
            for k in range(NS):
                st_s = stage.tile([P, D], BF16, tag="st", name="st_s")
                nc.gpsimd.dma_start(st_s[:], src_d[k * P : (k + 1) * P, :])
                nc.sync.dma_start_transpose(
                    srcT[:, :, k * P : (k + 1) * P], st_s[:]
                )
                st_w = stage.tile([P, D], BF16, tag="st", name="st_w")
                nc.gpsimd.dma_start(st_w[:], wq_d[k * P : (k + 1) * P, :])
                nc.sync.dma_start_transpose(
                    wqT[:, :, k * P : (k + 1) * P], st_w[:]
                )

            for qb in range(NQ):
                qs = stage.tile([P, D], BF16, tag="st", name="qs")
                nc.gpsimd.dma_start(qs[:], qry_d[qb * P : (qb + 1) * P, :])
                # qT[p, j, c] = query[qb*P+c, j*P+p]
                qT = qtp.tile([P, NJ, P], BF16, tag="qT", name="qT")
                nc.sync.dma_start_transpose(qT[:, :, :], qs[:])

                pss = [
                    ps1.tile([P, NB], F32, tag="ps", name="ps")
                    for _ in range(S // NB)
                ]
                for j in range(NJ):
                    lhsT = wkt[:, j, qb * P : (qb + 1) * P]
                    for sb in range(S // NB):
                        nc.tensor.matmul(
                            pss[sb],
                            lhsT,
                            srcT[:, j, sb * NB : (sb + 1) * NB],
                            start=(j == 0),
                            stop=False,
                        )
                for j in range(NJ):
                    lhsT = qT[:, j, :]
                    for sb in range(S // NB):
                        nc.tensor.matmul(
                            pss[sb],
                            lhsT,
                            wqT[:, j, sb * NB : (sb + 1) * NB],
                            start=False,
                            stop=(j == NJ - 1),
                        )
                for sb in range(S // NB):
                    nc.scalar.activation(
                        gt[:, qb, sb * NB : (sb + 1) * NB],
                        pss[sb],
                        AFT.Tanh,
                    )

            # kn fp32 -> bf16 DRAM bounce (emitted last: lowest DMA priority)
            for i in range(NS):
                kst = kpre.tile([P, T], BF16, tag="kst", name="kst")
                nc.gpsimd.dma_start(kst[:], kn_d[i * P : (i + 1) * P, :])
                nc.sync.dma_start(kn_bf[i * P : (i + 1) * P, :], kst[:])

        # ---------------- phase 2: sim = G @ kn, P = softmax(sim) ---------
        pm_pool = ctx.enter_context(tc.tile_pool(name="pm_pool", bufs=1))
        # P: [p, sc, t] = softmax(sim)[sc*P+p, t]  (s-part, t-free)
        pmat = pm_pool.tile([P, NS, T], BF16, name="pmat")

        with ExitStack() as ph2:
            res2 = ph2.enter_context(tc.tile_pool(name="res2", bufs=1))
            sm = ph2.enter_context(tc.tile_pool(name="sm", bufs=2))
            ps2p = ph2.enter_context(
                tc.tile_pool(name="ps2p", bufs=2, space="PSUM")
            )

            # knt[p, i, t] = kn[i*P+p, t]
            knt = res2.tile([P, NS, T], BF16, name="knt")
            for i in range(NS):
                nc.sync.dma_start(knt[:, i, :], kn_bf[i * P : (i + 1) * P, :])

            expv = res2.tile([P, T], F32, name="expv")
            for sc in range(NS):
                ps2 = ps2p.tile([P, T], F32, tag="ps2", name="ps2")
                for i in range(NS):
                    lhsT = gt[:, i, sc * P : (sc + 1) * P]
                    for tt in range(T // NB):
                        nc.tensor.matmul(
                            ps2[:, tt * NB : (tt + 1) * NB],
                            lhsT,
                            knt[:, i, tt * NB : (tt + 1) * NB],
                            start=(i == 0),
                            stop=(i == NS - 1),
                        )
                negmax = sm.tile([P, 1], F32, tag="negmax", name="negmax")
                nc.vector.reduce_max(negmax, ps2, axis=AX.X, negate=True)
                ssum = sm.tile([P, 1], F32, tag="ssum", name="ssum")
                nc.scalar.activation(
                    expv, ps2, AFT.Exp, bias=negmax, accum_out=ssum
                )
                rec = sm.tile([P, 1], F32, tag="rec", name="rec")
                nc.vector.reciprocal(rec, ssum)
                nc.vector.tensor_scalar_mul(pmat[:, sc, :], expv, rec)

        # ---------------- phase 3: out = P.T @ src -------------------------
        with ExitStack() as ph3:
            s3p = ph3.enter_context(tc.tile_pool(name="s3p", bufs=2))
            outp = ph3.enter_context(tc.tile_pool(name="outp", bufs=4))
            ps3p = ph3.enter_context(
                tc.tile_pool(name="ps3p", bufs=8, space="PSUM")
            )
            for dh in range(D // NB):
                # srcN[p, k, c] = src[k*P+p, dh*NB+c]
                srcN = s3p.tile([P, NS, NB], BF16, tag="srcN", name="srcN")
                for k in range(NS):
                    nc.gpsimd.dma_start(
                        srcN[:, k, :],
                        src_d[k * P : (k + 1) * P, dh * NB : (dh + 1) * NB],
                    )
                for tt in range(NT):
                    ps3 = ps3p.tile([P, NB], F32, tag="ps3", name="ps3")
                    for k in range(NS):
                        nc.tensor.matmul(
                            ps3,
                            pmat[:, k, tt * P : (tt + 1) * P],
                            srcN[:, k, :],
                            start=(k == 0),
                            stop=(k == NS - 1),
                        )
                    ot = outp.tile([P, NB], F32, tag="ot", name="ot")
                    nc.vector.tensor_copy(out=ot, in_=ps3)
                    nc.sync.dma_start(
                        out_d[tt * P : (tt + 1) * P, dh * NB : (dh + 1) * NB],
                        ot,
                    )


_NC_CACHE = None


def build_program():
    global _NC_CACHE
    if _NC_CACHE is not None:
        return _NC_CACHE
    nc = bacc.Bacc(
        "TRN2", target_bir_lowering=False, debug=False, num_devices=N_CORES
    )
    src_d = nc.dram_tensor("src", [S, D], F32, kind="ExternalInput").ap()
    qry_d = nc.dram_tensor("qry", [Q, D], F32, kind="ExternalInput").ap()
    kn_d = nc.dram_tensor("kn", [S, Q], F32, kind="ExternalInput").ap()
    wk_d = nc.dram_tensor("wk", [D, Q], F32, kind="ExternalInput").ap()
    wq_d = nc.dram_tensor("wq", [S, D], F32, kind="ExternalInput").ap()
    out_d = nc.dram_tensor("out", [Q, D], F32, kind="ExternalOutput").ap()
    with tile.TileContext(nc) as tc:
        _emit(tc, nc, src_d, qry_d, kn_d, wk_d, wq_d, out_d)
    nc.compile()
    _NC_CACHE = nc
    return nc


def make_in_maps(source, query, kernel, wk_kernel, wq_kernel):
    kn = np.ascontiguousarray(np.asarray(kernel, dtype=np.float32))
    wk = np.ascontiguousarray(np.asarray(wk_kernel, dtype=np.float32))
    wq = np.ascontiguousarray(np.asarray(wq_kernel, dtype=np.float32))
    return [
        {
            "src": np.ascontiguousarray(np.asarray(source[b], dtype=np.float32)),
            "qry": np.ascontiguousarray(np.asarray(query[b], dtype=np.float32)),
            "kn": kn,
            "wk": wk,
            "wq": wq,
        }
        for b in range(B)
    ]


def kernel(source, query, kernel, wk_kernel, wq_kernel):
    nc = build_program()
    in_maps = make_in_maps(source, query, kernel, wk_kernel, wq_kernel)
    res = bass_utils.run_bass_kernel_spmd(
        nc, in_maps, core_ids=list(range(N_CORES))
    )
    return np.stack([res.results[b]["out"] for b in range(B)], axis=0)


if __name__ == "__main__":
    rng = np.random.default_rng(0)
    ins = {
        "source": rng.standard_normal((B, S, D), dtype=np.float32),
        "query": rng.standard_normal((B, Q, D), dtype=np.float32),
        "kernel": rng.standard_normal((S, Q), dtype=np.float32) * 0.04,
        "wk_kernel": rng.standard_normal((D, Q), dtype=np.float32) * 0.04,
        "wq_kernel": rng.standard_normal((S, D), dtype=np.float32) * 0.04,
    }
    out = kernel(**ins)
    print(out.shape, out.dtype)
